# revision 18
# baseline (speedup 1.0000x reference)
"""Trainium2 Bass kernel for nn_Add_PairLinears.

y = sum_a( blockdiag2(W[a]) applied to x[:, perms[a]] ) + sum_a b[a]

Strategy (data-parallel over batch, 8 cores, no collectives):
  - Each core owns a batch shard of BC=1024 rows, processed full-width
    (x^T rows of 2KB) so SWDGE gather descriptors carry 2KB each --
    the gpsimd descriptor-generation cost (~2ns/descriptor measured)
    and the per-queue dispatch rate are the scaling limits, so fewer,
    fatter descriptors win.
  - Phase 1: load x [128b, 1024d] f32 tiles, PE-transpose the f32 data
    directly (2 cyc/row), convert to bf16 in the DVE PSUM->SBUF staging
    copy, stage j-groups [128, 8, BC] in SBUF, spill to DRAM x^T with
    2KB-per-partition-row descriptors. The staged tiles stay resident
    and serve as the identity mixer's (a=0) matmul operands.
  - lhsT loads are issued on the sync engine in the middle of phase 1's
    x loads so the head of the kernel gets full DMA bandwidth.
  - Gather: per output j-tile, one SWDGE dma_gather covering all 7
    permuted mixers (num_idxs=896, 2KB elements), round-robined over
    all 4 SWDGE queues.
  - Mix: per output j-tile, 8 accumulating 128x128 bf16 matmuls with
    full 1024-wide moving operands into [128, 1024] f32 PSUM (2 banks);
    evacuated with the per-partition bias sum_a b[a] fused (alternating
    scalar/DVE engines), stored as y^T bf16 with one batched DMA per
    2-j group on the scalar HWDGE queue.
"""

import os

import numpy as np
import ml_dtypes

import concourse.bass as bass
import concourse.bacc as bacc
import concourse.tile as tile
from concourse import library_config, mybir
from concourse.bass_utils import run_bass_kernel_spmd

B, D, A = 8192, 4096, 8
N_CORES = 8
BC = B // N_CORES          # 1024 batch rows per core
NJ = D // 128              # 32 d-tiles of 128
JSP = 8                    # j-tiles per phase-1 staging group
NSP = NJ // JSP            # staging groups (4)
JG = 1                     # j-tiles per mix group
NG = NJ // JG              # mix groups (32)
NQ = 4                     # SWDGE queues
JW = (A - 1) * 128 // 16   # idx columns per j-tile (56)

F32 = mybir.dt.float32
BF16 = mybir.dt.bfloat16
I16 = mybir.dt.int16

_GRAPH_CACHE = {}
_LAST_RESULTS = None


def _build_graph():
    nc = bacc.Bacc(None, num_swdge_queues=NQ, dynamic_dma_scratch_size=32768)

    x_ext = nc.declare_dram_parameter("x", [BC, D], F32, isOutput=False)
    lhsT_ext = nc.declare_dram_parameter("lhsT", [NJ, 128, A * 128], BF16, isOutput=False)
    idx_ext = nc.declare_dram_parameter("idx", [128, NJ * JW], I16, isOutput=False)
    bsum_ext = nc.declare_dram_parameter("bsum", [128, NJ], F32, isOutput=False)
    ident_ext = nc.declare_dram_parameter("ident", [128, 128], F32, isOutput=False)
    yt_ext = nc.declare_dram_parameter("yt", [D, BC], BF16, isOutput=True)

    qn = [0]

    def next_q():
        q = qn[0]
        qn[0] = (q + 1) % NQ
        return q

    with tile.TileContext(nc) as tc:
        with (
            tc.tile_pool(name="const", bufs=1) as constp,
            tc.tile_pool(name="xin", bufs=4) as xinp,
            tc.tile_pool(name="st", bufs=4) as stp,
            tc.tile_pool(name="lhs", bufs=3) as lhsp,
            tc.tile_pool(name="g", bufs=4) as gthp,
            tc.tile_pool(name="y", bufs=2) as ybp,
            tc.tile_pool(name="pst", bufs=2, space="PSUM") as pstp,
            tc.tile_pool(name="psm", bufs=4, space="PSUM") as psmp,
            tc.tile_pool(name="dram", bufs=1, space="DRAM") as dramp,
        ):
            nc.gpsimd.load_library(library_config.mlp)

            ident = constp.tile([128, 128], F32)
            nc.sync.dma_start(out=ident[:], in_=ident_ext[:])
            idx_sb = constp.tile([128, NJ * JW], I16)
            nc.scalar.dma_start(out=idx_sb[:], in_=idx_ext[:])
            bsum_sb = constp.tile([128, NJ], F32)
            nc.scalar.dma_start(out=bsum_sb[:], in_=bsum_ext[:])

            # lhsT streamed through a small ring on the sync HWDGE queue;
            # each 4-j chunk is consumed once, so no residency needed
            lhs_cur = [None]

            def load_lhs_chunk(j0):
                lhs_ch = lhsp.tile([128, 4, A * 128], BF16, tag="lhs")
                nc.sync.dma_start(
                    out=lhs_ch[:],
                    in_=lhsT_ext[j0:j0 + 4].rearrange("j t m -> t j m"))
                lhs_cur[0] = lhs_ch

            yt_v = yt_ext[:].rearrange("(j p) b -> p j b", p=128)

            # x^T DRAM buffer (full-width 2KB rows for the gathers)
            xt_d = dramp.tile([D, BC], BF16, tag="xt")
            xt_v = xt_d[:].rearrange("(q p) b -> p q b", p=128)

            # staged x^T tiles, kept resident for the identity mixer
            st_tiles = [None] * NSP

            def phase1_group(sg):
                """load + PE transpose (f32) for j-tiles [8sg, 8sg+8);
                stage in SBUF (bf16) and spill to DRAM x^T."""
                st = stp.tile([128, JSP, BC], BF16, tag="st")
                st_tiles[sg] = st
                for bt in range(BC // 128):
                    xin = xinp.tile([128, JSP * 128], F32, tag="xin")
                    nc.sync.dma_start(
                        out=xin[:],
                        in_=x_ext[bt * 128:(bt + 1) * 128,
                                  sg * JSP * 128:(sg + 1) * JSP * 128])
                    for jh in range(2):
                        pt = pstp.tile([128, 4, 128], F32, tag="pst")
                        for jq in range(4):
                            jj = jh * 4 + jq
                            nc.tensor.transpose(
                                pt[:, jq, :], xin[:, jj * 128:(jj + 1) * 128],
                                ident[:])
                        nc.vector.tensor_copy(
                            st[:, jh * 4:(jh + 1) * 4,
                               bt * 128:(bt + 1) * 128], pt[:])
                nc.scalar.dma_start(
                    out=xt_v[:, sg * JSP:(sg + 1) * JSP, :], in_=st[:])

            def mix_group(g):
                """gather (mixers 1..7) + mix + store for j-tiles
                [JG*g, JG*(g+1))."""
                gt = gthp.tile([128, JG * (A - 1), BC], BF16, tag="g")
                for jc in range(JG):
                    j = g * JG + jc
                    nc.gpsimd.dma_gather(
                        out_ap=gt[:, jc * (A - 1):(jc + 1) * (A - 1), :],
                        in_ap=xt_d[:],
                        idxs_ap=idx_sb[:, j * JW:(j + 1) * JW],
                        num_idxs=(A - 1) * 128,
                        num_idxs_reg=(A - 1) * 128,
                        elem_size=BC,
                        queue_num=next_q(),
                    )
                yb = ybp.tile([128, JG, BC], BF16, tag="y")
                for jc in range(JG):
                    j = g * JG + jc
                    if j % 4 == 0:
                        load_lhs_chunk(j)
                    lhs_ch = lhs_cur[0]
                    for bh in range(2):
                        bs = slice(bh * (BC // 2), (bh + 1) * (BC // 2))
                        pm = psmp.tile([128, BC // 2], F32, tag="psm")
                        for a in range(A):
                            if a == 0:
                                rhs = st_tiles[j // JSP][:, j % JSP, bs]
                            else:
                                rhs = gt[:, jc * (A - 1) + (a - 1), bs]
                            nc.tensor.matmul(
                                pm[:],
                                lhs_ch[:, j % 4, a * 128:(a + 1) * 128],
                                rhs,
                                start=(a == 0),
                                stop=(a == A - 1),
                            )
                        if (2 * (g * JG + jc) + bh) % 4 != 3:
                            nc.scalar.activation(
                                yb[:, jc, bs],
                                pm[:],
                                mybir.ActivationFunctionType.Identity,
                                bias=bsum_sb[:, j:j + 1],
                            )
                        else:
                            nc.vector.tensor_scalar_add(
                                yb[:, jc, bs], pm[:], bsum_sb[:, j:j + 1])
                nc.scalar.dma_start(
                    out=yt_v[:, g * JG:(g + 1) * JG, :], in_=yb[:])

            for sg in range(NSP):
                phase1_group(sg)
            for g in range(NG):
                mix_group(g)

    nc.compile()
    return nc


def _host_tables(W, b, perms):
    """Build the device-side constant tables from W/b/perms."""
    # lhsT[j, t, a, o]: weight applied to gathered row t (= x^T[perms[a, 128j+t]])
    # contributing to output row 128j+o.  Output 2n+oo uses inputs
    # perms[a, 2n+i] with weight W[a, n, i, oo]; within tile j, t = 2m+i,
    # o = 2m+oo for pair m = n - 64j.
    Wr = W.reshape(A, NJ, 64, 2, 2)
    lhsT = np.zeros((NJ, 128, A, 128), np.float32)
    m = np.arange(64)
    for i in range(2):
        for oo in range(2):
            # paired advanced indexing on axes 1 and 3 -> result axes [64, NJ, A]
            lhsT[:, 2 * m + i, :, 2 * m + oo] = Wr[:, :, :, i, oo].transpose(2, 1, 0)
    lhsT = np.ascontiguousarray(lhsT.reshape(NJ, 128, A * 128)).astype(ml_dtypes.bfloat16)

    # idx: per output j-tile, the concatenation over mixers a=1..7 of
    # perms[a, 128j : 128(j+1)], wrapped over 16 partitions (index i at
    # [i%16, i//16]) and replicated into each Q7 core's 16-partition group
    idx = np.zeros((128, NJ * JW), np.int16)
    for j in range(NJ):
        vec = np.concatenate([
            perms[a, j * 128:(j + 1) * 128] for a in range(1, A)
        ]).astype(np.int16)
        w16 = vec.reshape(JW, 16).T
        idx[:, j * JW:(j + 1) * JW] = np.tile(w16, (8, 1))

    bsum = np.ascontiguousarray(
        b.astype(np.float64).sum(axis=0).astype(np.float32).reshape(NJ, 128).T)
    ident = np.eye(128, dtype=np.float32)
    return lhsT, idx, bsum, ident


def kernel(x, W, b, perms):
    x = np.asarray(x, dtype=np.float32)
    W = np.asarray(W, dtype=np.float32)
    b = np.asarray(b, dtype=np.float32)
    perms = np.asarray(perms)

    lhsT, idx, bsum, ident = _host_tables(W, b, perms)

    if "nc" not in _GRAPH_CACHE:
        _GRAPH_CACHE["nc"] = _build_graph()
    nc = _GRAPH_CACHE["nc"]

    in_maps = []
    for c in range(N_CORES):
        in_maps.append({
            "x": np.ascontiguousarray(x[c * BC:(c + 1) * BC]),
            "lhsT": lhsT,
            "idx": idx,
            "bsum": bsum,
            "ident": ident,
        })

    res = run_bass_kernel_spmd(nc, in_maps, core_ids=list(range(N_CORES)))
    global _LAST_RESULTS
    _LAST_RESULTS = res
    y = np.concatenate(
        [np.asarray(res.results[c]["yt"], dtype=np.float32).T for c in range(N_CORES)],
        axis=0,
    )
    return np.ascontiguousarray(y)


# revision 20
# speedup vs baseline: 1.1105x; 1.1105x over previous
"""Trainium2 Bass kernel for nn_Add_PairLinears.

y = sum_a( blockdiag2(W[a]) applied to x[:, perms[a]] ) + sum_a b[a]

Strategy (data-parallel over batch, 8 cores, no collectives):
  - Each core owns a batch shard of BC=1024 rows, processed full-width
    (x^T rows of 2KB) so SWDGE gather descriptors carry 2KB each --
    the gpsimd descriptor-generation cost (~2ns/descriptor measured)
    and the per-queue dispatch rate are the scaling limits, so fewer,
    fatter descriptors win.
  - Phase 1: load x [128b, 1024d] f32 tiles, PE-transpose the f32 data
    directly (2 cyc/row), convert to bf16 in the DVE PSUM->SBUF staging
    copy, stage j-groups [128, 8, BC] in SBUF, spill to DRAM x^T with
    2KB-per-partition-row descriptors. The staged tiles stay resident
    and serve as the identity mixer's (a=0) matmul operands.
  - lhsT loads are issued on the sync engine in the middle of phase 1's
    x loads so the head of the kernel gets full DMA bandwidth.
  - Gather: per output j-tile, one SWDGE dma_gather covering all 7
    permuted mixers (num_idxs=896, 2KB elements), round-robined over
    all 4 SWDGE queues.
  - Mix: per output j-tile, 8 accumulating 128x128 bf16 matmuls with
    full 1024-wide moving operands into [128, 1024] f32 PSUM (2 banks);
    evacuated with the per-partition bias sum_a b[a] fused (alternating
    scalar/DVE engines), stored as y^T bf16 with one batched DMA per
    2-j group on the scalar HWDGE queue.
"""

import os

import numpy as np
import ml_dtypes

import concourse.bass as bass
import concourse.bacc as bacc
import concourse.tile as tile
from concourse import library_config, mybir
from concourse.bass_utils import run_bass_kernel_spmd

B, D, A = 8192, 4096, 8
N_CORES = 8
BC = B // N_CORES          # 1024 batch rows per core
NJ = D // 128              # 32 d-tiles of 128
JSP = 8                    # j-tiles per phase-1 staging group
NSP = NJ // JSP            # staging groups (4)
JG = 1                     # j-tiles per mix group
NG = NJ // JG              # mix groups (32)
NQ = 4                     # SWDGE queues
JW = (A - 1) * 128 // 16   # idx columns per j-tile (56)

F32 = mybir.dt.float32
BF16 = mybir.dt.bfloat16
I16 = mybir.dt.int16

_GRAPH_CACHE = {}
_LAST_RESULTS = None


def _build_graph():
    nc = bacc.Bacc(None, num_swdge_queues=NQ)

    x_ext = nc.declare_dram_parameter("x", [BC, D], F32, isOutput=False)
    lhsT_ext = nc.declare_dram_parameter("lhsT", [NJ, 128, A * 128], BF16, isOutput=False)
    idx_ext = nc.declare_dram_parameter("idx", [128, NJ * JW], I16, isOutput=False)
    bsum_ext = nc.declare_dram_parameter("bsum", [128, NJ], F32, isOutput=False)
    ident_ext = nc.declare_dram_parameter("ident", [128, 128], F32, isOutput=False)
    yt_ext = nc.declare_dram_parameter("yt", [D, BC], BF16, isOutput=True)

    qn = [0]

    def next_q():
        q = qn[0]
        qn[0] = (q + 1) % NQ
        return q

    with tile.TileContext(nc) as tc:
        with (
            tc.tile_pool(name="const", bufs=1) as constp,
            tc.tile_pool(name="xin", bufs=4) as xinp,
            tc.tile_pool(name="st", bufs=4) as stp,
            tc.tile_pool(name="lhs", bufs=3) as lhsp,
            tc.tile_pool(name="g", bufs=5) as gthp,
            tc.tile_pool(name="y", bufs=2) as ybp,
            tc.tile_pool(name="pst", bufs=2, space="PSUM") as pstp,
            tc.tile_pool(name="psm", bufs=6, space="PSUM") as psmp,
            tc.tile_pool(name="dram", bufs=1, space="DRAM") as dramp,
        ):
            nc.gpsimd.load_library(library_config.mlp)

            ident = constp.tile([128, 128], F32)
            nc.sync.dma_start(out=ident[:], in_=ident_ext[:])
            idx_sb = constp.tile([128, NJ * JW], I16)
            nc.scalar.dma_start(out=idx_sb[:], in_=idx_ext[:])
            bsum_sb = constp.tile([128, NJ], F32)
            nc.scalar.dma_start(out=bsum_sb[:], in_=bsum_ext[:])

            # lhsT streamed through a small ring on the sync HWDGE queue;
            # each 4-j chunk is consumed once, so no residency needed
            lhs_cur = [None]

            def load_lhs_chunk(j0):
                lhs_ch = lhsp.tile([128, 4, A * 128], BF16, tag="lhs")
                nc.sync.dma_start(
                    out=lhs_ch[:],
                    in_=lhsT_ext[j0:j0 + 4].rearrange("j t m -> t j m"))
                lhs_cur[0] = lhs_ch

            yt_v = yt_ext[:].rearrange("(j p) b -> p j b", p=128)

            # x^T DRAM buffer (full-width 2KB rows for the gathers)
            xt_d = dramp.tile([D, BC], BF16, tag="xt")
            xt_v = xt_d[:].rearrange("(q p) b -> p q b", p=128)

            # staged x^T tiles, kept resident for the identity mixer
            st_tiles = [None] * NSP

            def phase1_pair(hp):
                """load + PE transpose (f32) for j-tiles [16hp, 16hp+16)
                (two staging groups per 8KB-wide load); stage in SBUF (bf16)
                and spill to DRAM x^T."""
                st_a = stp.tile([128, JSP, BC], BF16, tag="st")
                st_b = stp.tile([128, JSP, BC], BF16, tag="st")
                sts = [st_a, st_b]
                st_tiles[2 * hp], st_tiles[2 * hp + 1] = sts
                for bt in range(BC // 128):
                    xin = xinp.tile([128, 2 * JSP * 128], F32, tag="xin")
                    nc.sync.dma_start(
                        out=xin[:],
                        in_=x_ext[bt * 128:(bt + 1) * 128,
                                  hp * 2048:(hp + 1) * 2048])
                    for jh in range(4):
                        pt = pstp.tile([128, 4, 128], F32, tag="pst")
                        for jq in range(4):
                            jj = jh * 4 + jq
                            nc.tensor.transpose(
                                pt[:, jq, :], xin[:, jj * 128:(jj + 1) * 128],
                                ident[:])
                        nc.vector.tensor_copy(
                            sts[jh // 2][:, (jh % 2) * 4:(jh % 2) * 4 + 4,
                                         bt * 128:(bt + 1) * 128], pt[:])
                for k in range(2):
                    sg = 2 * hp + k
                    nc.scalar.dma_start(
                        out=xt_v[:, sg * JSP:(sg + 1) * JSP, :],
                        in_=sts[k][:])

            def mix_group(g):
                """gather (mixers 1..7) + mix + store for j-tiles
                [JG*g, JG*(g+1))."""
                gt = gthp.tile([128, JG * (A - 1), BC], BF16, tag="g")
                for jc in range(JG):
                    j = g * JG + jc
                    nc.gpsimd.dma_gather(
                        out_ap=gt[:, jc * (A - 1):(jc + 1) * (A - 1), :],
                        in_ap=xt_d[:],
                        idxs_ap=idx_sb[:, j * JW:(j + 1) * JW],
                        num_idxs=(A - 1) * 128,
                        num_idxs_reg=(A - 1) * 128,
                        elem_size=BC,
                        queue_num=next_q(),
                    )
                yb = ybp.tile([128, JG, BC], BF16, tag="y")
                for jc in range(JG):
                    j = g * JG + jc
                    if j % 4 == 0:
                        load_lhs_chunk(j)
                    lhs_ch = lhs_cur[0]
                    for bh in range(2):
                        bs = slice(bh * (BC // 2), (bh + 1) * (BC // 2))
                        pm = psmp.tile([128, BC // 2], F32, tag="psm")
                        for a in range(A):
                            if a == 0:
                                rhs = st_tiles[j // JSP][:, j % JSP, bs]
                            else:
                                rhs = gt[:, jc * (A - 1) + (a - 1), bs]
                            nc.tensor.matmul(
                                pm[:],
                                lhs_ch[:, j % 4, a * 128:(a + 1) * 128],
                                rhs,
                                start=(a == 0),
                                stop=(a == A - 1),
                            )
                        if (2 * (g * JG + jc) + bh) % 4 != 3:
                            nc.scalar.activation(
                                yb[:, jc, bs],
                                pm[:],
                                mybir.ActivationFunctionType.Identity,
                                bias=bsum_sb[:, j:j + 1],
                            )
                        else:
                            nc.vector.tensor_scalar_add(
                                yb[:, jc, bs], pm[:], bsum_sb[:, j:j + 1])
                nc.scalar.dma_start(
                    out=yt_v[:, g * JG:(g + 1) * JG, :], in_=yb[:])

            for hp in range(NSP // 2):
                phase1_pair(hp)
            for g in range(NG):
                mix_group(g)

    nc.compile()
    return nc


def _host_tables(W, b, perms):
    """Build the device-side constant tables from W/b/perms."""
    # lhsT[j, t, a, o]: weight applied to gathered row t (= x^T[perms[a, 128j+t]])
    # contributing to output row 128j+o.  Output 2n+oo uses inputs
    # perms[a, 2n+i] with weight W[a, n, i, oo]; within tile j, t = 2m+i,
    # o = 2m+oo for pair m = n - 64j.
    Wr = W.reshape(A, NJ, 64, 2, 2)
    lhsT = np.zeros((NJ, 128, A, 128), np.float32)
    m = np.arange(64)
    for i in range(2):
        for oo in range(2):
            # paired advanced indexing on axes 1 and 3 -> result axes [64, NJ, A]
            lhsT[:, 2 * m + i, :, 2 * m + oo] = Wr[:, :, :, i, oo].transpose(2, 1, 0)
    lhsT = np.ascontiguousarray(lhsT.reshape(NJ, 128, A * 128)).astype(ml_dtypes.bfloat16)

    # idx: per output j-tile, the concatenation over mixers a=1..7 of
    # perms[a, 128j : 128(j+1)], wrapped over 16 partitions (index i at
    # [i%16, i//16]) and replicated into each Q7 core's 16-partition group
    idx = np.zeros((128, NJ * JW), np.int16)
    for j in range(NJ):
        vec = np.concatenate([
            perms[a, j * 128:(j + 1) * 128] for a in range(1, A)
        ]).astype(np.int16)
        w16 = vec.reshape(JW, 16).T
        idx[:, j * JW:(j + 1) * JW] = np.tile(w16, (8, 1))

    bsum = np.ascontiguousarray(
        b.astype(np.float64).sum(axis=0).astype(np.float32).reshape(NJ, 128).T)
    ident = np.eye(128, dtype=np.float32)
    return lhsT, idx, bsum, ident


def kernel(x, W, b, perms):
    x = np.asarray(x, dtype=np.float32)
    W = np.asarray(W, dtype=np.float32)
    b = np.asarray(b, dtype=np.float32)
    perms = np.asarray(perms)

    lhsT, idx, bsum, ident = _host_tables(W, b, perms)

    if "nc" not in _GRAPH_CACHE:
        _GRAPH_CACHE["nc"] = _build_graph()
    nc = _GRAPH_CACHE["nc"]

    in_maps = []
    for c in range(N_CORES):
        in_maps.append({
            "x": np.ascontiguousarray(x[c * BC:(c + 1) * BC]),
            "lhsT": lhsT,
            "idx": idx,
            "bsum": bsum,
            "ident": ident,
        })

    res = run_bass_kernel_spmd(nc, in_maps, core_ids=list(range(N_CORES)))
    global _LAST_RESULTS
    _LAST_RESULTS = res
    y = np.concatenate(
        [np.asarray(res.results[c]["yt"], dtype=np.float32).T for c in range(N_CORES)],
        axis=0,
    )
    return np.ascontiguousarray(y)


# revision 21
# speedup vs baseline: 1.2508x; 1.1263x over previous
"""Trainium2 Bass kernel for nn_Add_PairLinears.

y = sum_a( blockdiag2(W[a]) applied to x[:, perms[a]] ) + sum_a b[a]

Strategy (data-parallel over batch, 8 cores, no collectives):
  - Each core owns a batch shard of BC=1024 rows, processed full-width
    (x^T rows of 2KB) so SWDGE gather descriptors carry 2KB each --
    the gpsimd descriptor-generation cost (~2ns/descriptor measured)
    and the per-queue dispatch rate are the scaling limits, so fewer,
    fatter descriptors win.
  - Phase 1: load x [128b, 1024d] f32 tiles, PE-transpose the f32 data
    directly (2 cyc/row), convert to bf16 in the DVE PSUM->SBUF staging
    copy, stage j-groups [128, 8, BC] in SBUF, spill to DRAM x^T with
    2KB-per-partition-row descriptors. The staged tiles stay resident
    and serve as the identity mixer's (a=0) matmul operands.
  - lhsT loads are issued on the sync engine in the middle of phase 1's
    x loads so the head of the kernel gets full DMA bandwidth.
  - Gather: per output j-tile, one SWDGE dma_gather covering all 7
    permuted mixers (num_idxs=896, 2KB elements), round-robined over
    all 4 SWDGE queues.
  - Mix: per output j-tile, 8 accumulating 128x128 bf16 matmuls with
    full 1024-wide moving operands into [128, 1024] f32 PSUM (2 banks);
    evacuated with the per-partition bias sum_a b[a] fused (alternating
    scalar/DVE engines), stored as y^T bf16 with one batched DMA per
    2-j group on the scalar HWDGE queue.
"""

import os

import numpy as np
import ml_dtypes

import concourse.bass as bass
import concourse.bacc as bacc
import concourse.tile as tile
from concourse import library_config, mybir
from concourse.bass_utils import run_bass_kernel_spmd

B, D, A = 8192, 4096, 8
N_CORES = 8
BC = B // N_CORES          # 1024 batch rows per core
NJ = D // 128              # 32 d-tiles of 128
JSP = 8                    # j-tiles per phase-1 staging group
NSP = NJ // JSP            # staging groups (4)
JG = 1                     # j-tiles per mix group
NG = NJ // JG              # mix groups (32)
NQ = 4                     # SWDGE queues
JW = (A - 1) * 128 // 16   # idx columns per j-tile (56)

F32 = mybir.dt.float32
BF16 = mybir.dt.bfloat16
I16 = mybir.dt.int16

_GRAPH_CACHE = {}
_LAST_RESULTS = None


def _build_graph():
    nc = bacc.Bacc(None, num_swdge_queues=NQ)

    x_ext = nc.declare_dram_parameter("x", [BC, D], F32, isOutput=False)
    lhsT_ext = nc.declare_dram_parameter("lhsT", [NJ, 128, A * 128], BF16, isOutput=False)
    idx_ext = nc.declare_dram_parameter("idx", [128, NJ * JW], I16, isOutput=False)
    bsum_ext = nc.declare_dram_parameter("bsum", [128, NJ], F32, isOutput=False)
    ident_ext = nc.declare_dram_parameter("ident", [128, 128], F32, isOutput=False)
    yt_ext = nc.declare_dram_parameter("yt", [D, BC], BF16, isOutput=True)

    qn = [0]

    def next_q():
        q = qn[0]
        qn[0] = (q + 1) % NQ
        return q

    with tile.TileContext(nc) as tc:
        with (
            tc.tile_pool(name="const", bufs=1) as constp,
            tc.tile_pool(name="xin", bufs=3) as xinp,
            tc.tile_pool(name="st", bufs=4) as stp,
            tc.tile_pool(name="lhs", bufs=5) as lhsp,
            tc.tile_pool(name="g", bufs=4) as gthp,
            tc.tile_pool(name="y", bufs=4) as ybp,
            tc.tile_pool(name="pst", bufs=2, space="PSUM") as pstp,
            tc.tile_pool(name="psm", bufs=6, space="PSUM") as psmp,
            tc.tile_pool(name="dram", bufs=1, space="DRAM") as dramp,
        ):
            nc.gpsimd.load_library(library_config.mlp)

            ident = constp.tile([128, 128], F32)
            nc.sync.dma_start(out=ident[:], in_=ident_ext[:])
            idx_sb = constp.tile([128, NJ * JW], I16)
            nc.scalar.dma_start(out=idx_sb[:], in_=idx_ext[:])
            bsum_sb = constp.tile([128, NJ], F32)
            nc.scalar.dma_start(out=bsum_sb[:], in_=bsum_ext[:])

            # lhsT streamed through a small ring on the sync HWDGE queue;
            # each 4-j chunk is consumed once, so no residency needed
            lhs_cur = [None]

            def load_lhs_chunk(j0):
                lhs_ch = lhsp.tile([128, 4, A * 128], BF16, tag="lhs")
                nc.sync.dma_start(
                    out=lhs_ch[:],
                    in_=lhsT_ext[j0:j0 + 4].rearrange("j t m -> t j m"))
                lhs_cur[0] = lhs_ch

            yt_v = yt_ext[:].rearrange("(j p) b -> p j b", p=128)

            # x^T DRAM buffer (full-width 2KB rows for the gathers)
            xt_d = dramp.tile([D, BC], BF16, tag="xt")
            xt_v = xt_d[:].rearrange("(q p) b -> p q b", p=128)

            # staged x^T tiles, kept resident for the identity mixer
            st_tiles = [None] * NSP

            def phase1_pair(hp):
                """load + PE transpose (f32) for j-tiles [16hp, 16hp+16)
                (two staging groups per 8KB-wide load); stage in SBUF (bf16)
                and spill to DRAM x^T."""
                st_a = stp.tile([128, JSP, BC], BF16, tag="st")
                st_b = stp.tile([128, JSP, BC], BF16, tag="st")
                sts = [st_a, st_b]
                st_tiles[2 * hp], st_tiles[2 * hp + 1] = sts
                for bt in range(BC // 128):
                    xin = xinp.tile([128, 2 * JSP * 128], F32, tag="xin")
                    nc.sync.dma_start(
                        out=xin[:],
                        in_=x_ext[bt * 128:(bt + 1) * 128,
                                  hp * 2048:(hp + 1) * 2048])
                    for jh in range(4):
                        pt = pstp.tile([128, 4, 128], F32, tag="pst")
                        for jq in range(4):
                            jj = jh * 4 + jq
                            nc.tensor.transpose(
                                pt[:, jq, :], xin[:, jj * 128:(jj + 1) * 128],
                                ident[:])
                        nc.vector.tensor_copy(
                            sts[jh // 2][:, (jh % 2) * 4:(jh % 2) * 4 + 4,
                                         bt * 128:(bt + 1) * 128], pt[:])
                for k in range(2):
                    sg = 2 * hp + k
                    nc.scalar.dma_start(
                        out=xt_v[:, sg * JSP:(sg + 1) * JSP, :],
                        in_=sts[k][:])

            def mix_group(g):
                """gather (mixers 1..7) + mix + store for j-tiles
                [JG*g, JG*(g+1))."""
                gt = gthp.tile([128, JG * (A - 1), BC], BF16, tag="g")
                for jc in range(JG):
                    j = g * JG + jc
                    nc.gpsimd.dma_gather(
                        out_ap=gt[:, jc * (A - 1):(jc + 1) * (A - 1), :],
                        in_ap=xt_d[:],
                        idxs_ap=idx_sb[:, j * JW:(j + 1) * JW],
                        num_idxs=(A - 1) * 128,
                        num_idxs_reg=(A - 1) * 128,
                        elem_size=BC,
                        queue_num=next_q(),
                    )
                yb = ybp.tile([128, JG, BC], BF16, tag="y")
                for jc in range(JG):
                    j = g * JG + jc
                    if j % 4 == 0:
                        load_lhs_chunk(j)
                    lhs_ch = lhs_cur[0]
                    for bh in range(2):
                        bs = slice(bh * (BC // 2), (bh + 1) * (BC // 2))
                        pm = psmp.tile([128, BC // 2], F32, tag="psm")
                        for a in range(A):
                            if a == 0:
                                rhs = st_tiles[j // JSP][:, j % JSP, bs]
                            else:
                                rhs = gt[:, jc * (A - 1) + (a - 1), bs]
                            nc.tensor.matmul(
                                pm[:],
                                lhs_ch[:, j % 4, a * 128:(a + 1) * 128],
                                rhs,
                                start=(a == 0),
                                stop=(a == A - 1),
                            )
                        nc.scalar.activation(
                            yb[:, jc, bs],
                            pm[:],
                            mybir.ActivationFunctionType.Identity,
                            bias=bsum_sb[:, j:j + 1],
                        )
                nc.scalar.dma_start(
                    out=yt_v[:, g * JG:(g + 1) * JG, :], in_=yb[:])

            for hp in range(NSP // 2):
                phase1_pair(hp)
            for g in range(NG):
                mix_group(g)

    nc.compile()
    return nc


def _host_tables(W, b, perms):
    """Build the device-side constant tables from W/b/perms."""
    # lhsT[j, t, a, o]: weight applied to gathered row t (= x^T[perms[a, 128j+t]])
    # contributing to output row 128j+o.  Output 2n+oo uses inputs
    # perms[a, 2n+i] with weight W[a, n, i, oo]; within tile j, t = 2m+i,
    # o = 2m+oo for pair m = n - 64j.
    Wr = W.reshape(A, NJ, 64, 2, 2)
    lhsT = np.zeros((NJ, 128, A, 128), np.float32)
    m = np.arange(64)
    for i in range(2):
        for oo in range(2):
            # paired advanced indexing on axes 1 and 3 -> result axes [64, NJ, A]
            lhsT[:, 2 * m + i, :, 2 * m + oo] = Wr[:, :, :, i, oo].transpose(2, 1, 0)
    lhsT = np.ascontiguousarray(lhsT.reshape(NJ, 128, A * 128)).astype(ml_dtypes.bfloat16)

    # idx: per output j-tile, the concatenation over mixers a=1..7 of
    # perms[a, 128j : 128(j+1)], wrapped over 16 partitions (index i at
    # [i%16, i//16]) and replicated into each Q7 core's 16-partition group
    idx = np.zeros((128, NJ * JW), np.int16)
    for j in range(NJ):
        vec = np.concatenate([
            perms[a, j * 128:(j + 1) * 128] for a in range(1, A)
        ]).astype(np.int16)
        w16 = vec.reshape(JW, 16).T
        idx[:, j * JW:(j + 1) * JW] = np.tile(w16, (8, 1))

    bsum = np.ascontiguousarray(
        b.astype(np.float64).sum(axis=0).astype(np.float32).reshape(NJ, 128).T)
    ident = np.eye(128, dtype=np.float32)
    return lhsT, idx, bsum, ident


def kernel(x, W, b, perms):
    x = np.asarray(x, dtype=np.float32)
    W = np.asarray(W, dtype=np.float32)
    b = np.asarray(b, dtype=np.float32)
    perms = np.asarray(perms)

    lhsT, idx, bsum, ident = _host_tables(W, b, perms)

    if "nc" not in _GRAPH_CACHE:
        _GRAPH_CACHE["nc"] = _build_graph()
    nc = _GRAPH_CACHE["nc"]

    in_maps = []
    for c in range(N_CORES):
        in_maps.append({
            "x": np.ascontiguousarray(x[c * BC:(c + 1) * BC]),
            "lhsT": lhsT,
            "idx": idx,
            "bsum": bsum,
            "ident": ident,
        })

    res = run_bass_kernel_spmd(nc, in_maps, core_ids=list(range(N_CORES)))
    global _LAST_RESULTS
    _LAST_RESULTS = res
    y = np.concatenate(
        [np.asarray(res.results[c]["yt"], dtype=np.float32).T for c in range(N_CORES)],
        axis=0,
    )
    return np.ascontiguousarray(y)
